# revision 6
# baseline (speedup 1.0000x reference)
"""Causal self-attention Trainium2 Bass kernel (8 NeuronCores).

Problem: B=2, T=4096, C=512, H=8 heads, D=64 head dim.
  qkv = x @ w_attn.T + b_attn ; causal softmax attention ; y @ w_proj.T + b_proj

Sharding: 16 (batch, head) units over 8 cores -> core = b*4 + hp handles batch b
and heads 2hp, 2hp+1. Weights sliced per core on the host; each core emits a
[C, T] bf16 partial of the projected output for its head pair; the host sums
the 4 partials per batch (f32) and transposes back.

Design notes (v2):
- ACT (scalar engine) exp throughput is the hard floor (~1 elem/cycle/lane);
  the loop is arranged so ACT streams continuously: scores for two key-blocks
  are staged into one 2-bank PSUM tile and consumed by a single wide ACTIVATE
  (diag sub-blocks are packed contiguously so no garbage columns are read).
- All matmul lhsT operands span the full 128 partitions (per-head q is
  zero-padded into qT0/qT1) so LDWEIGHTS pipelines into the background weight
  buffer; partial-partition lhsT (row_grp) was measured to serialize
  LDWEIGHTS with the matmul stream and keep the PE HAM-throttled at 1.2 GHz.
- QKV projection is emitted just-in-time, interleaved between attention pairs
  (generator-driven) so there is no serial startup phase; x streams in as
  bf16 (halves HBM traffic).
- Softmax denominator rides as a ones-column in the P@V lhsT (M=65); the
  division avoids the DRAM bounce: evict O' to SBUF, matmul against a one-hot
  lhsT (e64) to broadcast the denominator row across 64 partitions,
  reciprocal, multiply.
- Both heads' normalized outputs are stacked into one [128, TQ] tile so the
  output projection runs with K=128 (half the matmuls); the result is stored
  as bf16 (halves the output DMA).
"""

import numpy as np

import concourse.bacc as bacc
import concourse.tile as tile
import concourse.mybir as mybir
from concourse import bass_utils
from concourse.bass import AP

F32 = mybir.dt.float32
F32R = mybir.dt.float32r
BF16 = mybir.dt.bfloat16
AF = mybir.ActivationFunctionType

B, T, C = 2, 4096, 512
H, D = 8, 64
N_CORES = 8
TQ = 512          # query tile
TJ = 128          # key block
NI = T // TQ      # 8 i-tiles
NJ = T // TJ      # 32 j-blocks

MM_DT = F32R      # attention-internal matmul dtype
IN_DT = BF16      # x / w_attn dtype (DMA-bound input path)


def _emit(nc, tc, ctx):
    xT = nc.dram_tensor("xT", [C, T], IN_DT, kind="ExternalInput").ap()
    wqkvT = nc.dram_tensor("wqkvT", [C, 384], IN_DT, kind="ExternalInput").ap()
    bqkv = nc.dram_tensor("bqkv", [128, 3], F32, kind="ExternalInput").ap()
    wpT = nc.dram_tensor("wpT", [128, C], MM_DT, kind="ExternalInput").ap()
    bp = nc.dram_tensor("bp", [128, 4], F32, kind="ExternalInput").ap()
    mask01 = nc.dram_tensor("mask01", [128, 128], F32, kind="ExternalInput").ap()
    ident = nc.dram_tensor("ident", [128, 128], F32, kind="ExternalInput").ap()
    e64 = nc.dram_tensor("e64", [128, 64], MM_DT, kind="ExternalInput").ap()
    outT = nc.dram_tensor("outT", [C, T], BF16, kind="ExternalOutput").ap()

    consts = ctx.enter_context(tc.tile_pool(name="consts", bufs=1))
    big = ctx.enter_context(tc.tile_pool(name="big", bufs=1))
    xt_pool = ctx.enter_context(tc.tile_pool(name="xt", bufs=8))
    vt_pool = ctx.enter_context(tc.tile_pool(name="vt", bufs=2))
    pt_pool = ctx.enter_context(tc.tile_pool(name="pt", bufs=5))
    osb_pool = ctx.enter_context(tc.tile_pool(name="osb", bufs=3))
    rc_pool = ctx.enter_context(tc.tile_pool(name="rc", bufs=3))
    yn_pool = ctx.enter_context(tc.tile_pool(name="yn", bufs=6))
    ob_pool = ctx.enter_context(tc.tile_pool(name="ob", bufs=4))
    # PSUM: 8 banks = stage 2x[128,1024] (4) + o 2x[128,512] (2) + sm 2x (2)
    ps_stage = ctx.enter_context(tc.tile_pool(name="ps_st", bufs=2, space="PSUM"))
    ps_o = ctx.enter_context(tc.tile_pool(name="ps_o", bufs=2, space="PSUM"))
    ps_sm = ctx.enter_context(tc.tile_pool(name="ps_sm", bufs=2, space="PSUM"))

    # --- constants ---
    w_sb = consts.tile([128, 4, 384], IN_DT, name="w_sb")
    nc.sync.dma_start(out=w_sb, in_=wqkvT.rearrange("(c p) m -> p c m", p=128))
    wp_sb = consts.tile([128, C], MM_DT, name="wp_sb")
    nc.sync.dma_start(out=wp_sb, in_=wpT)
    bqkv_sb = consts.tile([128, 3], F32, name="bqkv_sb")
    nc.sync.dma_start(out=bqkv_sb, in_=bqkv)
    bp_sb = consts.tile([128, 4], F32, name="bp_sb")
    nc.sync.dma_start(out=bp_sb, in_=bp)
    mask_sb = consts.tile([128, 128], F32, name="mask_sb")
    nc.sync.dma_start(out=mask_sb, in_=mask01)
    id_sb = consts.tile([128, 128], F32, name="id_sb")
    nc.sync.dma_start(out=id_sb, in_=ident)
    e64_sb = consts.tile([128, 64], MM_DT, name="e64_sb")
    nc.sync.dma_start(out=e64_sb, in_=e64)

    # per-head q (zero-padded to 128 partitions), shared k, natural-layout v
    qT0 = big.tile([128, T], MM_DT, name="qT0")
    qT1 = big.tile([128, T], MM_DT, name="qT1")
    kT_sb = big.tile([128, T], MM_DT, name="kT_sb")
    nc.vector.memset(qT0[64:128, :].bitcast(mybir.dt.uint32), 0)
    nc.gpsimd.memset(qT1[0:64, :].bitcast(mybir.dt.uint32), 0)
    # V per 128-key block with ones columns: [j, 0:64]=v_h0, 64=ones,
    # [65:129]=v_h1, 129=ones. memset, not DMA: a stride-0 broadcast DMA is
    # 4096 4-byte packets that hog the DMA queue for ~40us.
    v_all = big.tile([128, NJ, 130], MM_DT, name="v_all")
    one_bits = 0x3F800000
    nc.gpsimd.memset(v_all[:, :, 64:65].bitcast(mybir.dt.uint32), one_bits)
    nc.gpsimd.memset(v_all[:, :, 129:130].bitcast(mybir.dt.uint32), one_bits)

    # --- QKV projection for one i-tile (generator: yields between chunks) ---
    def emit_qkv(t):
        t0 = t * TQ
        xcs = []
        for c in range(4):
            xc = xt_pool.tile([128, TQ], IN_DT, name="xc", tag="xc")
            nc.sync.dma_start(out=xc, in_=xT[c * 128:(c + 1) * 128, t0:t0 + TQ])
            xcs.append(xc)
        yield
        for m in range(3):  # q, k, v
            ps = ps_sm.tile([128, TQ], F32, name="qkv_ps", tag="sm")
            for c in range(4):
                nc.tensor.matmul(
                    ps,
                    lhsT=w_sb[:, c, m * 128:(m + 1) * 128],
                    rhs=xcs[c],
                    start=(c == 0),
                    stop=(c == 3),
                )
            if m == 0:
                # q scale (1/sqrt(D)) folded into wqkvT/bqkv on the host
                nc.vector.tensor_scalar_add(
                    qT0[0:64, t0:t0 + TQ], ps[0:64, :], bqkv_sb[0:64, 0:1])
                nc.vector.tensor_scalar_add(
                    qT1[64:128, t0:t0 + TQ], ps[64:128, :], bqkv_sb[64:128, 0:1])
            elif m == 1:
                nc.vector.tensor_scalar_add(
                    kT_sb[:, t0:t0 + TQ], ps, bqkv_sb[:, 1:2])
            else:
                vt = vt_pool.tile([128, TQ], F32, name="vt", tag="vt")
                nc.vector.tensor_scalar_add(vt, ps, bqkv_sb[:, 2:3])
                for s in range(4):
                    n = t * 4 + s
                    tp = ps_sm.tile([128, 128], F32, name="tp", tag="sm")
                    # fp32 PE transpose (exact): [vdim, t]^T -> [t, vdim]
                    nc.tensor.transpose(tp, vt[:, s * 128:(s + 1) * 128], id_sb)
                    dst = v_all[:, n, :].rearrange(
                        "p (g e) -> p g e", g=2, e=65)[:, :, 0:64]
                    src = tp.rearrange("p (g e) -> p g e", g=2, e=64)
                    nc.vector.tensor_copy(dst, src)
            yield

    qkv_gens = [emit_qkv(t) for t in range(NI)]

    def drive(gen):
        if gen is not None:
            try:
                next(gen)
            except StopIteration:
                pass

    def finish(gen):
        if gen is not None:
            for _ in gen:
                pass

    finish(qkv_gens[0])  # i-tile 0 needed immediately

    # which QKV emission to interleave into each (half, h, i_t) block
    def gen_for(half, h, i_t):
        if half == 0 and h == 0 and i_t < 3:
            return qkv_gens[i_t + 1]
        if half == 0 and h == 1:
            return qkv_gens[4 + i_t]
        return None

    # --- attention: i_t-outer, J-pairs staged into one wide ACT ---
    pending_tail = [None]

    def flush_tail():
        if pending_tail[0] is not None:
            pending_tail[0]()
            pending_tail[0] = None

    yns = {}
    for half in range(2):
        for h in range(2):
            qT_h = qT0 if h == 0 else qT1
            for i_t in range(half * 4, half * 4 + 4):
                i0 = i_t * TQ
                nJ = 4 * i_t + 4
                gen = gen_for(half, h, i_t)
                o = ps_o.tile([128, TQ], F32, name="o_ps", tag="o")
                for Ja in range(0, nJ, 2):
                    drive(gen)
                    stage = ps_stage.tile([128, 1024], F32, name="st", tag="st")
                    metas, off = [], 0
                    for J in (Ja, Ja + 1):
                        r = max(0, J * TJ - i0)
                        w = TQ - r
                        nc.tensor.matmul(
                            stage[:, off:off + w],
                            lhsT=kT_sb[:, J * TJ:(J + 1) * TJ],
                            rhs=qT_h[:, i0 + r:i0 + TQ],
                            start=True, stop=True,
                        )
                        metas.append((J, r, off, w))
                        off += w
                    pt = pt_pool.tile([128, 1024], MM_DT, name="pt", tag="pt")
                    nc.scalar.activation(pt[:, 0:off], stage[:, 0:off], AF.Exp)
                    for (J, r, o_, w) in metas:
                        if J >= 4 * i_t:  # diag block: in-block triangle mask
                            nc.gpsimd.tensor_mul(
                                pt[:, o_:o_ + 128], pt[:, o_:o_ + 128], mask_sb)
                    for (J, r, o_, w) in metas:
                        v_lhs = (v_all[:, J, 0:65] if h == 0
                                 else v_all[:, J, 65:130])
                        nc.tensor.matmul(
                            o[0:65, r:TQ],
                            lhsT=v_lhs,
                            rhs=pt[:, o_:o_ + w],
                            start=(J == 0),
                            stop=(J == nJ - 1),
                        )
                    if Ja == 0:
                        flush_tail()
                finish(gen)

                def make_tail(h=h, i_t=i_t, o=o, i0=i0):
                    def tail():
                        # softmax division: broadcast denominator row via a
                        # one-hot matmul, reciprocal, multiply; evict O' from
                        # PSUM to SBUF first so the bank frees early.
                        o_sb = osb_pool.tile([128, TQ], MM_DT, name="o_sb",
                                             tag="osb")
                        nc.vector.tensor_copy(o_sb[0:65, :], o[0:65, :])
                        den = ps_sm.tile([128, TQ], F32, name="den", tag="sm")
                        nc.tensor.matmul(
                            den[0:64, :], lhsT=e64_sb[0:65, :],
                            rhs=o_sb[0:65, :], start=True, stop=True)
                        rc = rc_pool.tile([64, TQ], F32, name="rc", tag="rc")
                        nc.vector.reciprocal_approx_fast(out=rc, in_=den[0:64, :])
                        if h == 0:
                            yn = yn_pool.tile([128, TQ], MM_DT, name="yn",
                                              tag="yn")
                            yns[i_t] = yn
                        else:
                            yn = yns[i_t]
                        nc.vector.tensor_mul(
                            yn[h * 64:(h + 1) * 64, :], o_sb[0:64, :], rc)
                        if h == 1:
                            # output projection for this i_t (both heads)
                            for mc in range(4):
                                po = ps_sm.tile([128, TQ], F32, name="po",
                                                tag="sm")
                                nc.tensor.matmul(
                                    po, lhsT=wp_sb[:, mc * 128:(mc + 1) * 128],
                                    rhs=yn, start=True, stop=True)
                                ob = ob_pool.tile([128, TQ], BF16, name="ob",
                                                  tag="ob")
                                nc.vector.tensor_scalar_add(
                                    ob, po, bp_sb[:, mc:mc + 1])
                                nc.sync.dma_start(
                                    out=outT[mc * 128:(mc + 1) * 128,
                                             i0:i0 + TQ],
                                    in_=ob)
                    return tail

                pending_tail[0] = make_tail()
    flush_tail()


_CACHED_NC = None


def _build_program():
    global _CACHED_NC
    if _CACHED_NC is not None:
        return _CACHED_NC
    from contextlib import ExitStack
    nc = bacc.Bacc("TRN2", target_bir_lowering=False, debug=False,
                   num_devices=N_CORES)
    with tile.TileContext(nc) as tc:
        with ExitStack() as ctx:
            _emit(nc, tc, ctx)
    nc.compile()
    _CACHED_NC = nc
    return nc


def _host_inputs(x, w_attn, b_attn, w_proj, b_proj):
    """Build the 8 per-core input maps."""
    import ml_dtypes
    innp = ml_dtypes.bfloat16
    x = np.asarray(x, dtype=np.float32)
    w_attn = np.asarray(w_attn, dtype=np.float32)
    b_attn = np.asarray(b_attn, dtype=np.float32)
    w_proj = np.asarray(w_proj, dtype=np.float32)
    b_proj = np.asarray(b_proj, dtype=np.float32)

    scale = np.float32(1.0 / np.sqrt(D))
    mask = np.triu(np.ones((128, 128), dtype=np.float32))  # keep jj <= ii
    ident = np.eye(128, dtype=np.float32)
    e64 = np.zeros((128, 64), dtype=np.float32)
    e64[64, :] = 1.0

    xT_b = [np.ascontiguousarray(x[b].T).astype(innp) for b in range(B)]

    in_maps = []
    for core in range(N_CORES):
        b, hp = divmod(core, 4)
        r0 = 2 * hp * 64  # first row of this core's head-pair slice
        qr = w_attn[r0:r0 + 128] * scale
        kr = w_attn[C + r0:C + r0 + 128]
        vr = w_attn[2 * C + r0:2 * C + r0 + 128]
        wqkvT = np.ascontiguousarray(np.concatenate([qr, kr, vr], axis=0).T)
        bq = b_attn[r0:r0 + 128] * scale
        bk = b_attn[C + r0:C + r0 + 128]
        bv = b_attn[2 * C + r0:2 * C + r0 + 128]
        bqkv = np.ascontiguousarray(np.stack([bq, bk, bv], axis=1))
        wpT = np.ascontiguousarray(w_proj[:, r0:r0 + 128].T)
        if hp == 0:
            bpc = np.ascontiguousarray(b_proj.reshape(4, 128).T)
        else:
            bpc = np.zeros((128, 4), dtype=np.float32)
        in_maps.append({
            "xT": xT_b[b],
            "wqkvT": wqkvT.astype(innp),
            "bqkv": bqkv,
            "wpT": wpT.astype(np.float32),
            "bp": bpc,
            "mask01": mask,
            "ident": ident,
            "e64": e64,
        })
    return in_maps


def _gather(results):
    out = np.empty((B, T, C), dtype=np.float32)
    for b in range(B):
        acc = results[b * 4]["outT"].astype(np.float32)
        for hp in range(1, 4):
            acc = acc + results[b * 4 + hp]["outT"].astype(np.float32)
        out[b] = acc.T
    return out


def kernel(x, w_attn, b_attn, w_proj, b_proj, _run_kwargs=None):
    nc = _build_program()
    in_maps = _host_inputs(x, w_attn, b_attn, w_proj, b_proj)
    kw = dict(_run_kwargs or {})
    res = bass_utils.run_bass_kernel_spmd(nc, in_maps,
                                          core_ids=list(range(N_CORES)), **kw)
    out = _gather(res.results)
    if _run_kwargs is not None:
        kernel.last_result = res
    return out


# revision 7
# speedup vs baseline: 1.0004x; 1.0004x over previous
"""Causal self-attention Trainium2 Bass kernel (8 NeuronCores).

Problem: B=2, T=4096, C=512, H=8 heads, D=64 head dim.
  qkv = x @ w_attn.T + b_attn ; causal softmax attention ; y @ w_proj.T + b_proj

Sharding: 16 (batch, head) units over 8 cores -> core = b*4 + hp handles batch b
and heads 2hp, 2hp+1. Weights sliced per core on the host; each core emits a
[C, T] bf16 partial of the projected output for its head pair; the host sums
the 4 partials per batch (f32) and transposes back.

Design notes (v2):
- ACT (scalar engine) exp throughput is the hard floor (~1 elem/cycle/lane);
  the loop is arranged so ACT streams continuously: scores for two key-blocks
  are staged into one 2-bank PSUM tile and consumed by a single wide ACTIVATE
  (diag sub-blocks are packed contiguously so no garbage columns are read).
- All matmul lhsT operands span the full 128 partitions (per-head q is
  zero-padded into qT0/qT1) so LDWEIGHTS pipelines into the background weight
  buffer; partial-partition lhsT (row_grp) was measured to serialize
  LDWEIGHTS with the matmul stream and keep the PE HAM-throttled at 1.2 GHz.
- QKV projection is emitted just-in-time, interleaved between attention pairs
  (generator-driven) so there is no serial startup phase; x streams in as
  bf16 (halves HBM traffic).
- Softmax denominator rides as a ones-column in the P@V lhsT (M=65); the
  division avoids the DRAM bounce: evict O' to SBUF, matmul against a one-hot
  lhsT (e64) to broadcast the denominator row across 64 partitions,
  reciprocal, multiply.
- Both heads' normalized outputs are stacked into one [128, TQ] tile so the
  output projection runs with K=128 (half the matmuls); the result is stored
  as bf16 (halves the output DMA).
"""

import numpy as np

import concourse.bacc as bacc
import concourse.tile as tile
import concourse.mybir as mybir
from concourse import bass_utils
from concourse.bass import AP

F32 = mybir.dt.float32
F32R = mybir.dt.float32r
BF16 = mybir.dt.bfloat16
AF = mybir.ActivationFunctionType

B, T, C = 2, 4096, 512
H, D = 8, 64
N_CORES = 8
TQ = 512          # query tile
TJ = 128          # key block
NI = T // TQ      # 8 i-tiles
NJ = T // TJ      # 32 j-blocks

MM_DT = F32R      # attention-internal matmul dtype
IN_DT = BF16      # x / w_attn dtype (DMA-bound input path)


def _emit(nc, tc, ctx):
    xT = nc.dram_tensor("xT", [C, T], IN_DT, kind="ExternalInput").ap()
    wqkvT = nc.dram_tensor("wqkvT", [C, 384], IN_DT, kind="ExternalInput").ap()
    bqkv = nc.dram_tensor("bqkv", [128, 3], F32, kind="ExternalInput").ap()
    wpT = nc.dram_tensor("wpT", [128, C], MM_DT, kind="ExternalInput").ap()
    bp = nc.dram_tensor("bp", [128, 4], F32, kind="ExternalInput").ap()
    mask01 = nc.dram_tensor("mask01", [128, 128], F32, kind="ExternalInput").ap()
    ident = nc.dram_tensor("ident", [128, 128], F32, kind="ExternalInput").ap()
    e64 = nc.dram_tensor("e64", [128, 64], MM_DT, kind="ExternalInput").ap()
    outT = nc.dram_tensor("outT", [C, T], BF16, kind="ExternalOutput").ap()

    consts = ctx.enter_context(tc.tile_pool(name="consts", bufs=1))
    big = ctx.enter_context(tc.tile_pool(name="big", bufs=1))
    xt_pool = ctx.enter_context(tc.tile_pool(name="xt", bufs=8))
    vt_pool = ctx.enter_context(tc.tile_pool(name="vt", bufs=2))
    pt_pool = ctx.enter_context(tc.tile_pool(name="pt", bufs=3))
    osb_pool = ctx.enter_context(tc.tile_pool(name="osb", bufs=3))
    rc_pool = ctx.enter_context(tc.tile_pool(name="rc", bufs=3))
    yn_pool = ctx.enter_context(tc.tile_pool(name="yn", bufs=6))
    ob_pool = ctx.enter_context(tc.tile_pool(name="ob", bufs=4))
    # PSUM: 8 banks = stage 2x[128,1024] (4) + o 2x[128,512] (2) + sm 2x (2)
    ps_stage = ctx.enter_context(tc.tile_pool(name="ps_st", bufs=2, space="PSUM"))
    ps_o = ctx.enter_context(tc.tile_pool(name="ps_o", bufs=2, space="PSUM"))
    ps_sm = ctx.enter_context(tc.tile_pool(name="ps_sm", bufs=2, space="PSUM"))

    # --- constants ---
    w_sb = consts.tile([128, 4, 384], IN_DT, name="w_sb")
    nc.sync.dma_start(out=w_sb, in_=wqkvT.rearrange("(c p) m -> p c m", p=128))
    wp_sb = consts.tile([128, C], MM_DT, name="wp_sb")
    nc.sync.dma_start(out=wp_sb, in_=wpT)
    bqkv_sb = consts.tile([128, 3], F32, name="bqkv_sb")
    nc.sync.dma_start(out=bqkv_sb, in_=bqkv)
    bp_sb = consts.tile([128, 4], F32, name="bp_sb")
    nc.sync.dma_start(out=bp_sb, in_=bp)
    mask_sb = consts.tile([128, 128], F32, name="mask_sb")
    nc.sync.dma_start(out=mask_sb, in_=mask01)
    id_sb = consts.tile([128, 128], F32, name="id_sb")
    nc.sync.dma_start(out=id_sb, in_=ident)
    e64_sb = consts.tile([128, 64], MM_DT, name="e64_sb")
    nc.sync.dma_start(out=e64_sb, in_=e64)

    # per-head q (zero-padded to 128 partitions), shared k, natural-layout v
    qT0 = big.tile([128, T], MM_DT, name="qT0")
    qT1 = big.tile([128, T], MM_DT, name="qT1")
    kT_sb = big.tile([128, T], MM_DT, name="kT_sb")
    nc.vector.memset(qT0[64:128, :].bitcast(mybir.dt.uint32), 0)
    nc.gpsimd.memset(qT1[0:64, :].bitcast(mybir.dt.uint32), 0)
    # V per 128-key block with ones columns: [j, 0:64]=v_h0, 64=ones,
    # [65:129]=v_h1, 129=ones. memset, not DMA: a stride-0 broadcast DMA is
    # 4096 4-byte packets that hog the DMA queue for ~40us.
    v_all = big.tile([128, NJ, 130], MM_DT, name="v_all")
    one_bits = 0x3F800000
    nc.gpsimd.memset(v_all[:, :, 64:65].bitcast(mybir.dt.uint32), one_bits)
    nc.gpsimd.memset(v_all[:, :, 129:130].bitcast(mybir.dt.uint32), one_bits)

    # --- QKV projection for one i-tile (generator: yields between chunks) ---
    def emit_qkv(t):
        t0 = t * TQ
        xcs = []
        for c in range(4):
            xc = xt_pool.tile([128, TQ], IN_DT, name="xc", tag="xc")
            nc.sync.dma_start(out=xc, in_=xT[c * 128:(c + 1) * 128, t0:t0 + TQ])
            xcs.append(xc)
        yield
        for m in range(3):  # q, k, v
            ps = ps_sm.tile([128, TQ], F32, name="qkv_ps", tag="sm")
            for c in range(4):
                nc.tensor.matmul(
                    ps,
                    lhsT=w_sb[:, c, m * 128:(m + 1) * 128],
                    rhs=xcs[c],
                    start=(c == 0),
                    stop=(c == 3),
                )
            if m == 0:
                # q scale (1/sqrt(D)) folded into wqkvT/bqkv on the host
                nc.vector.tensor_scalar_add(
                    qT0[0:64, t0:t0 + TQ], ps[0:64, :], bqkv_sb[0:64, 0:1])
                nc.vector.tensor_scalar_add(
                    qT1[64:128, t0:t0 + TQ], ps[64:128, :], bqkv_sb[64:128, 0:1])
            elif m == 1:
                nc.vector.tensor_scalar_add(
                    kT_sb[:, t0:t0 + TQ], ps, bqkv_sb[:, 1:2])
            else:
                vt = vt_pool.tile([128, TQ], F32, name="vt", tag="vt")
                nc.vector.tensor_scalar_add(vt, ps, bqkv_sb[:, 2:3])
                for s in range(4):
                    n = t * 4 + s
                    tp = ps_sm.tile([128, 128], F32, name="tp", tag="sm")
                    # fp32 PE transpose (exact): [vdim, t]^T -> [t, vdim]
                    nc.tensor.transpose(tp, vt[:, s * 128:(s + 1) * 128], id_sb)
                    dst = v_all[:, n, :].rearrange(
                        "p (g e) -> p g e", g=2, e=65)[:, :, 0:64]
                    src = tp.rearrange("p (g e) -> p g e", g=2, e=64)
                    nc.vector.tensor_copy(dst, src)
            yield

    qkv_gens = [emit_qkv(t) for t in range(NI)]

    def drive(gen):
        if gen is not None:
            try:
                next(gen)
            except StopIteration:
                pass

    def finish(gen):
        if gen is not None:
            for _ in gen:
                pass

    finish(qkv_gens[0])  # i-tile 0 needed immediately

    # which QKV emission to interleave into each (half, h, i_t) block
    def gen_for(half, h, i_t):
        if half == 0 and h == 0 and i_t < 3:
            return qkv_gens[i_t + 1]
        if half == 0 and h == 1:
            return qkv_gens[4 + i_t]
        return None

    # --- attention: i_t-outer, J-pairs staged into one wide ACT ---
    pending_tail = [None]

    def flush_tail():
        if pending_tail[0] is not None:
            pending_tail[0]()
            pending_tail[0] = None

    yns = {}
    for half in range(2):
        for h in range(2):
            qT_h = qT0 if h == 0 else qT1
            for i_t in range(half * 4, half * 4 + 4):
                i0 = i_t * TQ
                nJ = 4 * i_t + 4
                gen = gen_for(half, h, i_t)
                o = ps_o.tile([128, TQ], F32, name="o_ps", tag="o")
                for Ja in range(0, nJ, 2):
                    drive(gen)
                    stage = ps_stage.tile([128, 1024], F32, name="st", tag="st")
                    metas, off = [], 0
                    for J in (Ja, Ja + 1):
                        r = max(0, J * TJ - i0)
                        w = TQ - r
                        nc.tensor.matmul(
                            stage[:, off:off + w],
                            lhsT=kT_sb[:, J * TJ:(J + 1) * TJ],
                            rhs=qT_h[:, i0 + r:i0 + TQ],
                            start=True, stop=True,
                        )
                        metas.append((J, r, off, w))
                        off += w
                    pt = pt_pool.tile([128, 1024], MM_DT, name="pt", tag="pt")
                    nc.scalar.activation(pt[:, 0:off], stage[:, 0:off], AF.Exp)
                    for (J, r, o_, w) in metas:
                        if J >= 4 * i_t:  # diag block: in-block triangle mask
                            nc.gpsimd.tensor_mul(
                                pt[:, o_:o_ + 128], pt[:, o_:o_ + 128], mask_sb)
                    for (J, r, o_, w) in metas:
                        v_lhs = (v_all[:, J, 0:65] if h == 0
                                 else v_all[:, J, 65:130])
                        nc.tensor.matmul(
                            o[0:65, r:TQ],
                            lhsT=v_lhs,
                            rhs=pt[:, o_:o_ + w],
                            start=(J == 0),
                            stop=(J == nJ - 1),
                        )
                    if Ja == 0:
                        flush_tail()
                finish(gen)

                def make_tail(h=h, i_t=i_t, o=o, i0=i0):
                    def tail():
                        # softmax division: broadcast denominator row via a
                        # one-hot matmul, reciprocal, multiply; evict O' from
                        # PSUM to SBUF first so the bank frees early.
                        o_sb = osb_pool.tile([128, TQ], MM_DT, name="o_sb",
                                             tag="osb")
                        nc.vector.tensor_copy(o_sb[0:65, :], o[0:65, :])
                        den = ps_sm.tile([128, TQ], F32, name="den", tag="sm")
                        nc.tensor.matmul(
                            den[0:64, :], lhsT=e64_sb[0:65, :],
                            rhs=o_sb[0:65, :], start=True, stop=True)
                        rc = rc_pool.tile([64, TQ], F32, name="rc", tag="rc")
                        nc.vector.reciprocal_approx_fast(out=rc, in_=den[0:64, :])
                        if h == 0:
                            yn = yn_pool.tile([128, TQ], MM_DT, name="yn",
                                              tag="yn")
                            yns[i_t] = yn
                        else:
                            yn = yns[i_t]
                        nc.vector.tensor_mul(
                            yn[h * 64:(h + 1) * 64, :], o_sb[0:64, :], rc)
                        if h == 1:
                            # output projection for this i_t (both heads)
                            for mc in range(4):
                                po = ps_sm.tile([128, TQ], F32, name="po",
                                                tag="sm")
                                nc.tensor.matmul(
                                    po, lhsT=wp_sb[:, mc * 128:(mc + 1) * 128],
                                    rhs=yn, start=True, stop=True)
                                ob = ob_pool.tile([128, TQ], BF16, name="ob",
                                                  tag="ob")
                                nc.vector.tensor_scalar_add(
                                    ob, po, bp_sb[:, mc:mc + 1])
                                nc.sync.dma_start(
                                    out=outT[mc * 128:(mc + 1) * 128,
                                             i0:i0 + TQ],
                                    in_=ob)
                    return tail

                pending_tail[0] = make_tail()
    flush_tail()


_CACHED_NC = None


def _build_program():
    global _CACHED_NC
    if _CACHED_NC is not None:
        return _CACHED_NC
    from contextlib import ExitStack
    nc = bacc.Bacc("TRN2", target_bir_lowering=False, debug=False,
                   num_devices=N_CORES)
    with tile.TileContext(nc) as tc:
        with ExitStack() as ctx:
            _emit(nc, tc, ctx)
    nc.compile()
    _CACHED_NC = nc
    return nc


def _host_inputs(x, w_attn, b_attn, w_proj, b_proj):
    """Build the 8 per-core input maps."""
    import ml_dtypes
    innp = ml_dtypes.bfloat16
    x = np.asarray(x, dtype=np.float32)
    w_attn = np.asarray(w_attn, dtype=np.float32)
    b_attn = np.asarray(b_attn, dtype=np.float32)
    w_proj = np.asarray(w_proj, dtype=np.float32)
    b_proj = np.asarray(b_proj, dtype=np.float32)

    scale = np.float32(1.0 / np.sqrt(D))
    mask = np.triu(np.ones((128, 128), dtype=np.float32))  # keep jj <= ii
    ident = np.eye(128, dtype=np.float32)
    e64 = np.zeros((128, 64), dtype=np.float32)
    e64[64, :] = 1.0

    xT_b = [np.ascontiguousarray(x[b].T).astype(innp) for b in range(B)]

    in_maps = []
    for core in range(N_CORES):
        b, hp = divmod(core, 4)
        r0 = 2 * hp * 64  # first row of this core's head-pair slice
        qr = w_attn[r0:r0 + 128] * scale
        kr = w_attn[C + r0:C + r0 + 128]
        vr = w_attn[2 * C + r0:2 * C + r0 + 128]
        wqkvT = np.ascontiguousarray(np.concatenate([qr, kr, vr], axis=0).T)
        bq = b_attn[r0:r0 + 128] * scale
        bk = b_attn[C + r0:C + r0 + 128]
        bv = b_attn[2 * C + r0:2 * C + r0 + 128]
        bqkv = np.ascontiguousarray(np.stack([bq, bk, bv], axis=1))
        wpT = np.ascontiguousarray(w_proj[:, r0:r0 + 128].T)
        if hp == 0:
            bpc = np.ascontiguousarray(b_proj.reshape(4, 128).T)
        else:
            bpc = np.zeros((128, 4), dtype=np.float32)
        in_maps.append({
            "xT": xT_b[b],
            "wqkvT": wqkvT.astype(innp),
            "bqkv": bqkv,
            "wpT": wpT.astype(np.float32),
            "bp": bpc,
            "mask01": mask,
            "ident": ident,
            "e64": e64,
        })
    return in_maps


def _gather(results):
    out = np.empty((B, T, C), dtype=np.float32)
    for b in range(B):
        acc = results[b * 4]["outT"].astype(np.float32)
        for hp in range(1, 4):
            acc = acc + results[b * 4 + hp]["outT"].astype(np.float32)
        out[b] = acc.T
    return out


def kernel(x, w_attn, b_attn, w_proj, b_proj, _run_kwargs=None):
    nc = _build_program()
    in_maps = _host_inputs(x, w_attn, b_attn, w_proj, b_proj)
    kw = dict(_run_kwargs or {})
    res = bass_utils.run_bass_kernel_spmd(nc, in_maps,
                                          core_ids=list(range(N_CORES)), **kw)
    out = _gather(res.results)
    if _run_kwargs is not None:
        kernel.last_result = res
    return out


# revision 10
# speedup vs baseline: 1.1715x; 1.1710x over previous
"""Causal self-attention Trainium2 Bass kernel (8 NeuronCores).

Problem: B=2, T=4096, C=512, H=8 heads, D=64 head dim.
  qkv = x @ w_attn.T + b_attn ; causal softmax attention ; y @ w_proj.T + b_proj

Sharding: 16 (batch, head) units over 8 cores -> core = b*4 + hp handles batch b
and heads 2hp, 2hp+1. Weights sliced per core on the host; each core emits a
[C, T] bf16 partial of the projected output for its head pair; the host sums
the 4 partials per batch (f32) and transposes back.

Design notes (v2):
- ACT (scalar engine) exp throughput is the hard floor (~1 elem/cycle/lane);
  the loop is arranged so ACT streams continuously: scores for two key-blocks
  are staged into one 2-bank PSUM tile and consumed by a single wide ACTIVATE
  (diag sub-blocks are packed contiguously so no garbage columns are read).
- All matmul lhsT operands span the full 128 partitions (per-head q is
  zero-padded into qT0/qT1) so LDWEIGHTS pipelines into the background weight
  buffer; partial-partition lhsT (row_grp) was measured to serialize
  LDWEIGHTS with the matmul stream and keep the PE HAM-throttled at 1.2 GHz.
- QKV projection is emitted just-in-time, interleaved between attention pairs
  (generator-driven) so there is no serial startup phase; x streams in as
  bf16 (halves HBM traffic).
- Softmax denominator rides as a ones-column in the P@V lhsT (M=65); the
  division avoids the DRAM bounce: evict O' to SBUF, matmul against a one-hot
  lhsT (e64) to broadcast the denominator row across 64 partitions,
  reciprocal, multiply.
- Both heads' normalized outputs are stacked into one [128, TQ] tile so the
  output projection runs with K=128 (half the matmuls); the result is stored
  as bf16 (halves the output DMA).
"""

import numpy as np

import concourse.bacc as bacc
import concourse.tile as tile
import concourse.mybir as mybir
from concourse import bass_utils
from concourse.bass import AP

F32 = mybir.dt.float32
F32R = mybir.dt.float32r
BF16 = mybir.dt.bfloat16
AF = mybir.ActivationFunctionType

B, T, C = 2, 4096, 512
H, D = 8, 64
N_CORES = 8
TQ = 512          # query tile
TJ = 128          # key block
NI = T // TQ      # 8 i-tiles
NJ = T // TJ      # 32 j-blocks

MM_DT = F32R      # attention-internal matmul dtype
IN_DT = BF16      # x / w_attn dtype (DMA-bound input path)


def _emit(nc, tc, ctx):
    xT = nc.dram_tensor("xT", [C, T], IN_DT, kind="ExternalInput").ap()
    wqkvT = nc.dram_tensor("wqkvT", [C, 384], IN_DT, kind="ExternalInput").ap()
    bqkv = nc.dram_tensor("bqkv", [128, 3], F32, kind="ExternalInput").ap()
    wpT = nc.dram_tensor("wpT", [128, C], MM_DT, kind="ExternalInput").ap()
    bp = nc.dram_tensor("bp", [128, 4], F32, kind="ExternalInput").ap()
    mask01 = nc.dram_tensor("mask01", [128, 128], F32, kind="ExternalInput").ap()
    ident = nc.dram_tensor("ident", [128, 128], F32, kind="ExternalInput").ap()
    e64 = nc.dram_tensor("e64", [128, 64], MM_DT, kind="ExternalInput").ap()
    outT = nc.dram_tensor("outT", [C, T], BF16, kind="ExternalOutput").ap()

    consts = ctx.enter_context(tc.tile_pool(name="consts", bufs=1))
    big = ctx.enter_context(tc.tile_pool(name="big", bufs=1))
    vt_pool = ctx.enter_context(tc.tile_pool(name="vt", bufs=2))
    pt_pool = ctx.enter_context(tc.tile_pool(name="pt", bufs=3))
    osb_pool = ctx.enter_context(tc.tile_pool(name="osb", bufs=3))
    rc_pool = ctx.enter_context(tc.tile_pool(name="rc", bufs=3))
    yn_pool = ctx.enter_context(tc.tile_pool(name="yn", bufs=6))
    ob_pool = ctx.enter_context(tc.tile_pool(name="ob", bufs=4))
    # PSUM: 8 banks = stage 2x[128,1024] (4) + o 2x[128,512] (2) + sm 2x (2)
    ps_stage = ctx.enter_context(tc.tile_pool(name="ps_st", bufs=2, space="PSUM"))
    ps_o = ctx.enter_context(tc.tile_pool(name="ps_o", bufs=2, space="PSUM"))
    ps_sm = ctx.enter_context(tc.tile_pool(name="ps_sm", bufs=2, space="PSUM"))

    # --- constants ---
    w_sb = consts.tile([128, 4, 384], IN_DT, name="w_sb")
    nc.sync.dma_start(out=w_sb, in_=wqkvT.rearrange("(c p) m -> p c m", p=128))
    wp_sb = consts.tile([128, C], MM_DT, name="wp_sb")
    nc.sync.dma_start(out=wp_sb, in_=wpT)
    bqkv_sb = consts.tile([128, 3], F32, name="bqkv_sb")
    nc.sync.dma_start(out=bqkv_sb, in_=bqkv)
    bp_sb = consts.tile([128, 4], F32, name="bp_sb")
    nc.sync.dma_start(out=bp_sb, in_=bp)
    mask_sb = consts.tile([128, 128], F32, name="mask_sb")
    nc.sync.dma_start(out=mask_sb, in_=mask01)
    id_sb = consts.tile([128, 128], F32, name="id_sb")
    nc.sync.dma_start(out=id_sb, in_=ident)
    e64_sb = consts.tile([128, 64], MM_DT, name="e64_sb")
    nc.sync.dma_start(out=e64_sb, in_=e64)

    # x resident in SBUF via a few big DMAs issued up-front: streaming x
    # during the attention phase inflates every SBUF access ~20% (measured),
    # so the input burst is confined to the cheap QKV ramp.
    xfull = big.tile([128, 4, T], IN_DT, name="xfull")
    for cc in range(4):
        nc.sync.dma_start(out=xfull[:, cc, 0:2048],
                          in_=xT[cc * 128:(cc + 1) * 128, 0:2048])
    for cc in range(4):
        nc.sync.dma_start(out=xfull[:, cc, 2048:T],
                          in_=xT[cc * 128:(cc + 1) * 128, 2048:T])

    # per-head q (zero-padded to 128 partitions), shared k, natural-layout v
    qT0 = big.tile([128, T], MM_DT, name="qT0")
    qT1 = big.tile([128, T], MM_DT, name="qT1")
    kT_sb = big.tile([128, T], MM_DT, name="kT_sb")
    nc.vector.memset(qT0[64:128, :].bitcast(mybir.dt.uint32), 0)
    nc.gpsimd.memset(qT1[0:64, :].bitcast(mybir.dt.uint32), 0)
    # V per 128-key block with ones columns: [j, 0:64]=v_h0, 64=ones,
    # [65:129]=v_h1, 129=ones. memset, not DMA: a stride-0 broadcast DMA is
    # 4096 4-byte packets that hog the DMA queue for ~40us.
    v_all = big.tile([128, NJ, 130], MM_DT, name="v_all")
    one_bits = 0x3F800000
    nc.gpsimd.memset(v_all[:, :, 64:65].bitcast(mybir.dt.uint32), one_bits)
    nc.gpsimd.memset(v_all[:, :, 129:130].bitcast(mybir.dt.uint32), one_bits)

    # --- QKV projection for one i-tile (generator: yields between chunks) ---
    def emit_qkv(t):
        t0 = t * TQ
        for m in range(3):  # q, k, v
            ps = ps_sm.tile([128, TQ], F32, name="qkv_ps", tag="sm")
            for c in range(4):
                nc.tensor.matmul(
                    ps,
                    lhsT=w_sb[:, c, m * 128:(m + 1) * 128],
                    rhs=xfull[:, c, t0:t0 + TQ],
                    start=(c == 0),
                    stop=(c == 3),
                )
            if m == 0:
                # q scale (1/sqrt(D)) folded into wqkvT/bqkv on the host
                nc.vector.tensor_scalar_add(
                    qT0[0:64, t0:t0 + TQ], ps[0:64, :], bqkv_sb[0:64, 0:1])
                nc.vector.tensor_scalar_add(
                    qT1[64:128, t0:t0 + TQ], ps[64:128, :], bqkv_sb[64:128, 0:1])
            elif m == 1:
                nc.vector.tensor_scalar_add(
                    kT_sb[:, t0:t0 + TQ], ps, bqkv_sb[:, 1:2])
            else:
                vt = vt_pool.tile([128, TQ], F32, name="vt", tag="vt")
                nc.vector.tensor_scalar_add(vt, ps, bqkv_sb[:, 2:3])
                for s in range(4):
                    n = t * 4 + s
                    tp = ps_sm.tile([128, 128], F32, name="tp", tag="sm")
                    # fp32 PE transpose (exact): [vdim, t]^T -> [t, vdim]
                    nc.tensor.transpose(tp, vt[:, s * 128:(s + 1) * 128], id_sb)
                    dst = v_all[:, n, :].rearrange(
                        "p (g e) -> p g e", g=2, e=65)[:, :, 0:64]
                    src = tp.rearrange("p (g e) -> p g e", g=2, e=64)
                    nc.vector.tensor_copy(dst, src)
            yield

    qkv_gens = [emit_qkv(t) for t in range(NI)]

    def drive(gen):
        if gen is not None:
            try:
                next(gen)
            except StopIteration:
                pass

    def finish(gen):
        if gen is not None:
            for _ in gen:
                pass

    finish(qkv_gens[0])  # i-tile 0 needed immediately

    # which QKV emission to interleave into each (half, h, i_t) block
    def gen_for(half, h, i_t):
        if half == 0 and h == 0 and i_t < 3:
            return qkv_gens[i_t + 1]
        if half == 0 and h == 1:
            return qkv_gens[4 + i_t]
        return None

    # --- attention: i_t-outer, J-pairs staged into one wide ACT ---
    pending_tail = [None]

    def flush_tail():
        if pending_tail[0] is not None:
            pending_tail[0]()
            pending_tail[0] = None

    yns = {}
    for half in range(2):
        for h in range(2):
            qT_h = qT0 if h == 0 else qT1
            for i_t in range(half * 4, half * 4 + 4):
                i0 = i_t * TQ
                nJ = 4 * i_t + 4
                gen = gen_for(half, h, i_t)
                o = ps_o.tile([128, TQ], F32, name="o_ps", tag="o")
                for Ja in range(0, nJ, 2):
                    drive(gen)
                    stage = ps_stage.tile([128, 1024], F32, name="st", tag="st")
                    metas, off = [], 0
                    for J in (Ja, Ja + 1):
                        r = max(0, J * TJ - i0)
                        w = TQ - r
                        nc.tensor.matmul(
                            stage[:, off:off + w],
                            lhsT=kT_sb[:, J * TJ:(J + 1) * TJ],
                            rhs=qT_h[:, i0 + r:i0 + TQ],
                            start=True, stop=True,
                        )
                        metas.append((J, r, off, w))
                        off += w
                    pt = pt_pool.tile([128, 1024], MM_DT, name="pt", tag="pt")
                    nc.scalar.activation(pt[:, 0:off], stage[:, 0:off], AF.Exp)
                    for (J, r, o_, w) in metas:
                        if J >= 4 * i_t:  # diag block: in-block triangle mask
                            nc.gpsimd.tensor_mul(
                                pt[:, o_:o_ + 128], pt[:, o_:o_ + 128], mask_sb)
                    for (J, r, o_, w) in metas:
                        v_lhs = (v_all[:, J, 0:65] if h == 0
                                 else v_all[:, J, 65:130])
                        nc.tensor.matmul(
                            o[0:65, r:TQ],
                            lhsT=v_lhs,
                            rhs=pt[:, o_:o_ + w],
                            start=(J == 0),
                            stop=(J == nJ - 1),
                        )
                    if Ja == 0:
                        flush_tail()
                finish(gen)

                def make_tail(h=h, i_t=i_t, o=o, i0=i0):
                    def tail():
                        # softmax division: broadcast denominator row via a
                        # one-hot matmul, reciprocal, multiply; evict O' from
                        # PSUM to SBUF first so the bank frees early.
                        o_sb = osb_pool.tile([128, TQ], MM_DT, name="o_sb",
                                             tag="osb")
                        nc.vector.tensor_copy(o_sb[0:65, :], o[0:65, :])
                        den = ps_sm.tile([128, TQ], F32, name="den", tag="sm")
                        nc.tensor.matmul(
                            den[0:64, :], lhsT=e64_sb[0:65, :],
                            rhs=o_sb[0:65, :], start=True, stop=True)
                        rc = rc_pool.tile([64, TQ], F32, name="rc", tag="rc")
                        nc.vector.reciprocal_approx_fast(out=rc, in_=den[0:64, :])
                        if h == 0:
                            yn = yn_pool.tile([128, TQ], MM_DT, name="yn",
                                              tag="yn")
                            yns[i_t] = yn
                        else:
                            yn = yns[i_t]
                        nc.vector.tensor_mul(
                            yn[h * 64:(h + 1) * 64, :], o_sb[0:64, :], rc)
                        if h == 1:
                            # output projection for this i_t (both heads)
                            for mc in range(4):
                                po = ps_sm.tile([128, TQ], F32, name="po",
                                                tag="sm")
                                nc.tensor.matmul(
                                    po, lhsT=wp_sb[:, mc * 128:(mc + 1) * 128],
                                    rhs=yn, start=True, stop=True)
                                ob = ob_pool.tile([128, TQ], BF16, name="ob",
                                                  tag="ob")
                                nc.vector.tensor_scalar_add(
                                    ob, po, bp_sb[:, mc:mc + 1])
                                nc.sync.dma_start(
                                    out=outT[mc * 128:(mc + 1) * 128,
                                             i0:i0 + TQ],
                                    in_=ob)
                    return tail

                pending_tail[0] = make_tail()
    flush_tail()


_CACHED_NC = None


def _build_program():
    global _CACHED_NC
    if _CACHED_NC is not None:
        return _CACHED_NC
    from contextlib import ExitStack
    nc = bacc.Bacc("TRN2", target_bir_lowering=False, debug=False,
                   num_devices=N_CORES)
    with tile.TileContext(nc) as tc:
        with ExitStack() as ctx:
            _emit(nc, tc, ctx)
    nc.compile()
    _CACHED_NC = nc
    return nc


def _host_inputs(x, w_attn, b_attn, w_proj, b_proj):
    """Build the 8 per-core input maps."""
    import ml_dtypes
    innp = ml_dtypes.bfloat16
    x = np.asarray(x, dtype=np.float32)
    w_attn = np.asarray(w_attn, dtype=np.float32)
    b_attn = np.asarray(b_attn, dtype=np.float32)
    w_proj = np.asarray(w_proj, dtype=np.float32)
    b_proj = np.asarray(b_proj, dtype=np.float32)

    scale = np.float32(1.0 / np.sqrt(D))
    mask = np.triu(np.ones((128, 128), dtype=np.float32))  # keep jj <= ii
    ident = np.eye(128, dtype=np.float32)
    e64 = np.zeros((128, 64), dtype=np.float32)
    e64[64, :] = 1.0

    xT_b = [np.ascontiguousarray(x[b].T).astype(innp) for b in range(B)]

    in_maps = []
    for core in range(N_CORES):
        b, hp = divmod(core, 4)
        r0 = 2 * hp * 64  # first row of this core's head-pair slice
        qr = w_attn[r0:r0 + 128] * scale
        kr = w_attn[C + r0:C + r0 + 128]
        vr = w_attn[2 * C + r0:2 * C + r0 + 128]
        wqkvT = np.ascontiguousarray(np.concatenate([qr, kr, vr], axis=0).T)
        bq = b_attn[r0:r0 + 128] * scale
        bk = b_attn[C + r0:C + r0 + 128]
        bv = b_attn[2 * C + r0:2 * C + r0 + 128]
        bqkv = np.ascontiguousarray(np.stack([bq, bk, bv], axis=1))
        wpT = np.ascontiguousarray(w_proj[:, r0:r0 + 128].T)
        if hp == 0:
            bpc = np.ascontiguousarray(b_proj.reshape(4, 128).T)
        else:
            bpc = np.zeros((128, 4), dtype=np.float32)
        in_maps.append({
            "xT": xT_b[b],
            "wqkvT": wqkvT.astype(innp),
            "bqkv": bqkv,
            "wpT": wpT.astype(np.float32),
            "bp": bpc,
            "mask01": mask,
            "ident": ident,
            "e64": e64,
        })
    return in_maps


def _gather(results):
    out = np.empty((B, T, C), dtype=np.float32)
    for b in range(B):
        acc = results[b * 4]["outT"].astype(np.float32)
        for hp in range(1, 4):
            acc = acc + results[b * 4 + hp]["outT"].astype(np.float32)
        out[b] = acc.T
    return out


def kernel(x, w_attn, b_attn, w_proj, b_proj, _run_kwargs=None):
    nc = _build_program()
    in_maps = _host_inputs(x, w_attn, b_attn, w_proj, b_proj)
    kw = dict(_run_kwargs or {})
    res = bass_utils.run_bass_kernel_spmd(nc, in_maps,
                                          core_ids=list(range(N_CORES)), **kw)
    out = _gather(res.results)
    if _run_kwargs is not None:
        kernel.last_result = res
    return out


# revision 11
# speedup vs baseline: 1.2139x; 1.0362x over previous
"""Causal self-attention Trainium2 Bass kernel (8 NeuronCores).

Problem: B=2, T=4096, C=512, H=8 heads, D=64 head dim.
  qkv = x @ w_attn.T + b_attn ; causal softmax attention ; y @ w_proj.T + b_proj

Sharding: 16 (batch, head) units over 8 cores -> core = b*4 + hp handles batch b
and heads 2hp, 2hp+1. Weights sliced per core on the host; each core emits a
[C, T] bf16 partial of the projected output for its head pair; the host sums
the 4 partials per batch (f32) and transposes back.

Design notes (v2):
- ACT (scalar engine) exp throughput is the hard floor (~1 elem/cycle/lane);
  the loop is arranged so ACT streams continuously: scores for two key-blocks
  are staged into one 2-bank PSUM tile and consumed by a single wide ACTIVATE
  (diag sub-blocks are packed contiguously so no garbage columns are read).
- All matmul lhsT operands span the full 128 partitions (per-head q is
  zero-padded into qT0/qT1) so LDWEIGHTS pipelines into the background weight
  buffer; partial-partition lhsT (row_grp) was measured to serialize
  LDWEIGHTS with the matmul stream and keep the PE HAM-throttled at 1.2 GHz.
- QKV projection is emitted just-in-time, interleaved between attention pairs
  (generator-driven) so there is no serial startup phase; x streams in as
  bf16 (halves HBM traffic).
- Softmax denominator rides as a ones-column in the P@V lhsT (M=65); the
  division avoids the DRAM bounce: evict O' to SBUF, matmul against a one-hot
  lhsT (e64) to broadcast the denominator row across 64 partitions,
  reciprocal, multiply.
- Both heads' normalized outputs are stacked into one [128, TQ] tile so the
  output projection runs with K=128 (half the matmuls); the result is stored
  as bf16 (halves the output DMA).
"""

import numpy as np

import concourse.bacc as bacc
import concourse.tile as tile
import concourse.mybir as mybir
from concourse import bass_utils
from concourse.bass import AP

F32 = mybir.dt.float32
F32R = mybir.dt.float32r
BF16 = mybir.dt.bfloat16
AF = mybir.ActivationFunctionType

B, T, C = 2, 4096, 512
H, D = 8, 64
N_CORES = 8
TQ = 512          # query tile
TJ = 128          # key block
NI = T // TQ      # 8 i-tiles
NJ = T // TJ      # 32 j-blocks

MM_DT = F32R      # attention-internal matmul dtype
IN_DT = BF16      # x / w_attn dtype (DMA-bound input path)


def _emit(nc, tc, ctx):
    xT = nc.dram_tensor("xT", [C, T], IN_DT, kind="ExternalInput").ap()
    wqkvT = nc.dram_tensor("wqkvT", [C, 384], IN_DT, kind="ExternalInput").ap()
    bqkv = nc.dram_tensor("bqkv", [128, 3], F32, kind="ExternalInput").ap()
    wpT = nc.dram_tensor("wpT", [128, C], MM_DT, kind="ExternalInput").ap()
    bp = nc.dram_tensor("bp", [128, 4], F32, kind="ExternalInput").ap()
    mask01 = nc.dram_tensor("mask01", [128, 128], F32, kind="ExternalInput").ap()
    ident = nc.dram_tensor("ident", [128, 128], F32, kind="ExternalInput").ap()
    e64 = nc.dram_tensor("e64", [128, 64], MM_DT, kind="ExternalInput").ap()
    outT = nc.dram_tensor("outT", [C, T], BF16, kind="ExternalOutput").ap()

    consts = ctx.enter_context(tc.tile_pool(name="consts", bufs=1))
    big = ctx.enter_context(tc.tile_pool(name="big", bufs=1))
    vt_pool = ctx.enter_context(tc.tile_pool(name="vt", bufs=2))
    pt_pool = ctx.enter_context(tc.tile_pool(name="pt", bufs=4))
    osb_pool = ctx.enter_context(tc.tile_pool(name="osb", bufs=3))
    rc_pool = ctx.enter_context(tc.tile_pool(name="rc", bufs=3))
    yn_pool = ctx.enter_context(tc.tile_pool(name="yn", bufs=6))
    ob_pool = ctx.enter_context(tc.tile_pool(name="ob", bufs=4))
    # PSUM: 8 banks = stage 2x[128,1024] (4) + o 2x[128,512] (2) + sm 2x (2)
    ps_stage = ctx.enter_context(tc.tile_pool(name="ps_st", bufs=2, space="PSUM"))
    ps_o = ctx.enter_context(tc.tile_pool(name="ps_o", bufs=2, space="PSUM"))
    ps_sm = ctx.enter_context(tc.tile_pool(name="ps_sm", bufs=2, space="PSUM"))

    # --- constants ---
    w_sb = consts.tile([128, 4, 384], IN_DT, name="w_sb")
    nc.sync.dma_start(out=w_sb, in_=wqkvT.rearrange("(c p) m -> p c m", p=128))
    wp_sb = consts.tile([128, C], MM_DT, name="wp_sb")
    nc.sync.dma_start(out=wp_sb, in_=wpT)
    bqkv_sb = consts.tile([128, 3], F32, name="bqkv_sb")
    nc.sync.dma_start(out=bqkv_sb, in_=bqkv)
    bp_sb = consts.tile([128, 4], F32, name="bp_sb")
    nc.sync.dma_start(out=bp_sb, in_=bp)
    mask_sb = consts.tile([128, 128], F32, name="mask_sb")
    nc.sync.dma_start(out=mask_sb, in_=mask01)
    id_sb = consts.tile([128, 128], F32, name="id_sb")
    nc.sync.dma_start(out=id_sb, in_=ident)
    e64_sb = consts.tile([128, 64], MM_DT, name="e64_sb")
    nc.sync.dma_start(out=e64_sb, in_=e64)

    # x resident in SBUF via a few big DMAs issued up-front: streaming x
    # during the attention phase inflates every SBUF access ~20% (measured),
    # so the input burst is confined to the cheap QKV ramp.
    xfull = big.tile([128, 4, T], IN_DT, name="xfull")
    for cc in range(4):
        nc.sync.dma_start(out=xfull[:, cc, 0:2048],
                          in_=xT[cc * 128:(cc + 1) * 128, 0:2048])
    for cc in range(4):
        nc.sync.dma_start(out=xfull[:, cc, 2048:T],
                          in_=xT[cc * 128:(cc + 1) * 128, 2048:T])

    # per-head q (zero-padded to 128 partitions), shared k, natural-layout v
    qT0 = big.tile([128, T], MM_DT, name="qT0")
    qT1 = big.tile([128, T], MM_DT, name="qT1")
    kT_sb = big.tile([128, T], MM_DT, name="kT_sb")
    nc.vector.memset(qT0[64:128, :].bitcast(mybir.dt.uint32), 0)
    nc.gpsimd.memset(qT1[0:64, :].bitcast(mybir.dt.uint32), 0)
    # V per 128-key block with ones columns: [j, 0:64]=v_h0, 64=ones,
    # [65:129]=v_h1, 129=ones. memset, not DMA: a stride-0 broadcast DMA is
    # 4096 4-byte packets that hog the DMA queue for ~40us.
    v_all = big.tile([128, NJ, 130], MM_DT, name="v_all")
    one_bits = 0x3F800000
    nc.gpsimd.memset(v_all[:, :, 64:65].bitcast(mybir.dt.uint32), one_bits)
    nc.gpsimd.memset(v_all[:, :, 129:130].bitcast(mybir.dt.uint32), one_bits)

    # --- QKV projection for one i-tile (generator: yields between chunks) ---
    def emit_qkv(t):
        t0 = t * TQ
        for m in range(3):  # q, k, v
            ps = ps_sm.tile([128, TQ], F32, name="qkv_ps", tag="sm")
            for c in range(4):
                nc.tensor.matmul(
                    ps,
                    lhsT=w_sb[:, c, m * 128:(m + 1) * 128],
                    rhs=xfull[:, c, t0:t0 + TQ],
                    start=(c == 0),
                    stop=(c == 3),
                )
            if m == 0:
                # q scale (1/sqrt(D)) folded into wqkvT/bqkv on the host
                nc.vector.tensor_scalar_add(
                    qT0[0:64, t0:t0 + TQ], ps[0:64, :], bqkv_sb[0:64, 0:1])
                nc.vector.tensor_scalar_add(
                    qT1[64:128, t0:t0 + TQ], ps[64:128, :], bqkv_sb[64:128, 0:1])
            elif m == 1:
                nc.vector.tensor_scalar_add(
                    kT_sb[:, t0:t0 + TQ], ps, bqkv_sb[:, 1:2])
            else:
                vt = vt_pool.tile([128, TQ], F32, name="vt", tag="vt")
                nc.vector.tensor_scalar_add(vt, ps, bqkv_sb[:, 2:3])
                for s in range(4):
                    n = t * 4 + s
                    tp = ps_sm.tile([128, 128], F32, name="tp", tag="sm")
                    # fp32 PE transpose (exact): [vdim, t]^T -> [t, vdim]
                    nc.tensor.transpose(tp, vt[:, s * 128:(s + 1) * 128], id_sb)
                    dst = v_all[:, n, :].rearrange(
                        "p (g e) -> p g e", g=2, e=65)[:, :, 0:64]
                    src = tp.rearrange("p (g e) -> p g e", g=2, e=64)
                    nc.vector.tensor_copy(dst, src)
            yield

    qkv_gens = [emit_qkv(t) for t in range(NI)]

    def drive(gen):
        if gen is not None:
            try:
                next(gen)
            except StopIteration:
                pass

    def finish(gen):
        if gen is not None:
            for _ in gen:
                pass

    finish(qkv_gens[0])  # i-tile 0 needed immediately

    # which QKV emission to interleave into each (half, h, i_t) block
    def gen_for(half, h, i_t):
        if half == 0 and h == 0 and i_t < 3:
            return qkv_gens[i_t + 1]
        if half == 0 and h == 1:
            return qkv_gens[4 + i_t]
        return None

    # --- attention: i_t-outer, J-pairs staged into one wide ACT ---
    pending_tail = [None]

    def flush_tail():
        if pending_tail[0] is not None:
            pending_tail[0]()
            pending_tail[0] = None

    yns = {}
    for half in range(2):
        for h in range(2):
            qT_h = qT0 if h == 0 else qT1
            for i_t in range(half * 4, half * 4 + 4):
                i0 = i_t * TQ
                nJ = 4 * i_t + 4
                gen = gen_for(half, h, i_t)
                o = ps_o.tile([128, TQ], F32, name="o_ps", tag="o")
                for Ja in range(0, nJ, 2):
                    drive(gen)
                    stage = ps_stage.tile([128, 1024], F32, name="st", tag="st")
                    metas, off = [], 0
                    for J in (Ja, Ja + 1):
                        r = max(0, J * TJ - i0)
                        w = TQ - r
                        nc.tensor.matmul(
                            stage[:, off:off + w],
                            lhsT=kT_sb[:, J * TJ:(J + 1) * TJ],
                            rhs=qT_h[:, i0 + r:i0 + TQ],
                            start=True, stop=True,
                        )
                        metas.append((J, r, off, w))
                        off += w
                    pt = pt_pool.tile([128, 1024], MM_DT, name="pt", tag="pt")
                    nc.scalar.activation(pt[:, 0:off], stage[:, 0:off], AF.Exp)
                    for (J, r, o_, w) in metas:
                        if J >= 4 * i_t:  # diag block: in-block triangle mask
                            nc.vector.tensor_mul(
                                pt[:, o_:o_ + 128], pt[:, o_:o_ + 128], mask_sb)
                    for (J, r, o_, w) in metas:
                        v_lhs = (v_all[:, J, 0:65] if h == 0
                                 else v_all[:, J, 65:130])
                        nc.tensor.matmul(
                            o[0:65, r:TQ],
                            lhsT=v_lhs,
                            rhs=pt[:, o_:o_ + w],
                            start=(J == 0),
                            stop=(J == nJ - 1),
                        )
                    if Ja == 0:
                        flush_tail()
                finish(gen)

                def make_tail(h=h, i_t=i_t, o=o, i0=i0):
                    def tail():
                        # softmax division: broadcast denominator row via a
                        # one-hot matmul, reciprocal, multiply; evict O' from
                        # PSUM to SBUF first so the bank frees early.
                        o_sb = osb_pool.tile([128, TQ], MM_DT, name="o_sb",
                                             tag="osb")
                        nc.vector.tensor_copy(o_sb[0:65, :], o[0:65, :])
                        den = ps_sm.tile([128, TQ], F32, name="den", tag="sm")
                        nc.tensor.matmul(
                            den[0:64, :], lhsT=e64_sb[0:65, :],
                            rhs=o_sb[0:65, :], start=True, stop=True)
                        rc = rc_pool.tile([64, TQ], F32, name="rc", tag="rc")
                        nc.vector.reciprocal_approx_fast(out=rc, in_=den[0:64, :])
                        if h == 0:
                            yn = yn_pool.tile([128, TQ], MM_DT, name="yn",
                                              tag="yn")
                            yns[i_t] = yn
                        else:
                            yn = yns[i_t]
                        nc.vector.tensor_mul(
                            yn[h * 64:(h + 1) * 64, :], o_sb[0:64, :], rc)
                        if h == 1:
                            # output projection for this i_t (both heads)
                            for mc in range(4):
                                po = ps_sm.tile([128, TQ], F32, name="po",
                                                tag="sm")
                                nc.tensor.matmul(
                                    po, lhsT=wp_sb[:, mc * 128:(mc + 1) * 128],
                                    rhs=yn, start=True, stop=True)
                                ob = ob_pool.tile([128, TQ], BF16, name="ob",
                                                  tag="ob")
                                nc.vector.tensor_scalar_add(
                                    ob, po, bp_sb[:, mc:mc + 1])
                                nc.sync.dma_start(
                                    out=outT[mc * 128:(mc + 1) * 128,
                                             i0:i0 + TQ],
                                    in_=ob)
                    return tail

                pending_tail[0] = make_tail()
    flush_tail()


_CACHED_NC = None


def _build_program():
    global _CACHED_NC
    if _CACHED_NC is not None:
        return _CACHED_NC
    from contextlib import ExitStack
    nc = bacc.Bacc("TRN2", target_bir_lowering=False, debug=False,
                   num_devices=N_CORES)
    with tile.TileContext(nc) as tc:
        with ExitStack() as ctx:
            _emit(nc, tc, ctx)
    nc.compile()
    _CACHED_NC = nc
    return nc


def _host_inputs(x, w_attn, b_attn, w_proj, b_proj):
    """Build the 8 per-core input maps."""
    import ml_dtypes
    innp = ml_dtypes.bfloat16
    x = np.asarray(x, dtype=np.float32)
    w_attn = np.asarray(w_attn, dtype=np.float32)
    b_attn = np.asarray(b_attn, dtype=np.float32)
    w_proj = np.asarray(w_proj, dtype=np.float32)
    b_proj = np.asarray(b_proj, dtype=np.float32)

    scale = np.float32(1.0 / np.sqrt(D))
    mask = np.triu(np.ones((128, 128), dtype=np.float32))  # keep jj <= ii
    ident = np.eye(128, dtype=np.float32)
    e64 = np.zeros((128, 64), dtype=np.float32)
    e64[64, :] = 1.0

    xT_b = [np.ascontiguousarray(x[b].T).astype(innp) for b in range(B)]

    in_maps = []
    for core in range(N_CORES):
        b, hp = divmod(core, 4)
        r0 = 2 * hp * 64  # first row of this core's head-pair slice
        qr = w_attn[r0:r0 + 128] * scale
        kr = w_attn[C + r0:C + r0 + 128]
        vr = w_attn[2 * C + r0:2 * C + r0 + 128]
        wqkvT = np.ascontiguousarray(np.concatenate([qr, kr, vr], axis=0).T)
        bq = b_attn[r0:r0 + 128] * scale
        bk = b_attn[C + r0:C + r0 + 128]
        bv = b_attn[2 * C + r0:2 * C + r0 + 128]
        bqkv = np.ascontiguousarray(np.stack([bq, bk, bv], axis=1))
        wpT = np.ascontiguousarray(w_proj[:, r0:r0 + 128].T)
        if hp == 0:
            bpc = np.ascontiguousarray(b_proj.reshape(4, 128).T)
        else:
            bpc = np.zeros((128, 4), dtype=np.float32)
        in_maps.append({
            "xT": xT_b[b],
            "wqkvT": wqkvT.astype(innp),
            "bqkv": bqkv,
            "wpT": wpT.astype(np.float32),
            "bp": bpc,
            "mask01": mask,
            "ident": ident,
            "e64": e64,
        })
    return in_maps


def _gather(results):
    out = np.empty((B, T, C), dtype=np.float32)
    for b in range(B):
        acc = results[b * 4]["outT"].astype(np.float32)
        for hp in range(1, 4):
            acc = acc + results[b * 4 + hp]["outT"].astype(np.float32)
        out[b] = acc.T
    return out


def kernel(x, w_attn, b_attn, w_proj, b_proj, _run_kwargs=None):
    nc = _build_program()
    in_maps = _host_inputs(x, w_attn, b_attn, w_proj, b_proj)
    kw = dict(_run_kwargs or {})
    res = bass_utils.run_bass_kernel_spmd(nc, in_maps,
                                          core_ids=list(range(N_CORES)), **kw)
    out = _gather(res.results)
    if _run_kwargs is not None:
        kernel.last_result = res
    return out


# revision 12
# speedup vs baseline: 1.2697x; 1.0459x over previous
"""Causal self-attention Trainium2 Bass kernel (8 NeuronCores).

Problem: B=2, T=4096, C=512, H=8 heads, D=64 head dim.
  qkv = x @ w_attn.T + b_attn ; causal softmax attention ; y @ w_proj.T + b_proj

Sharding: 16 (batch, head) units over 8 cores -> core = b*4 + hp handles batch b
and heads 2hp, 2hp+1. Weights sliced per core on the host; each core emits a
[C, T] bf16 partial of the projected output for its head pair; the host sums
the 4 partials per batch (f32) and transposes back.

Design notes (v2):
- ACT (scalar engine) exp throughput is the hard floor (~1 elem/cycle/lane);
  the loop is arranged so ACT streams continuously: scores for two key-blocks
  are staged into one 2-bank PSUM tile and consumed by a single wide ACTIVATE
  (diag sub-blocks are packed contiguously so no garbage columns are read).
- All matmul lhsT operands span the full 128 partitions (per-head q is
  zero-padded into qT0/qT1) so LDWEIGHTS pipelines into the background weight
  buffer; partial-partition lhsT (row_grp) was measured to serialize
  LDWEIGHTS with the matmul stream and keep the PE HAM-throttled at 1.2 GHz.
- QKV projection is emitted just-in-time, interleaved between attention pairs
  (generator-driven) so there is no serial startup phase; x streams in as
  bf16 (halves HBM traffic).
- Softmax denominator rides as a ones-column in the P@V lhsT (M=65); the
  division avoids the DRAM bounce: evict O' to SBUF, matmul against a one-hot
  lhsT (e64) to broadcast the denominator row across 64 partitions,
  reciprocal, multiply.
- Both heads' normalized outputs are stacked into one [128, TQ] tile so the
  output projection runs with K=128 (half the matmuls); the result is stored
  as bf16 (halves the output DMA).
"""

import numpy as np

import concourse.bacc as bacc
import concourse.tile as tile
import concourse.mybir as mybir
from concourse import bass_utils
from concourse.bass import AP

F32 = mybir.dt.float32
F32R = mybir.dt.float32r
BF16 = mybir.dt.bfloat16
AF = mybir.ActivationFunctionType

B, T, C = 2, 4096, 512
H, D = 8, 64
N_CORES = 8
TQ = 512          # query tile
TJ = 128          # key block
NI = T // TQ      # 8 i-tiles
NJ = T // TJ      # 32 j-blocks

MM_DT = F32R      # attention-internal matmul dtype
IN_DT = BF16      # x / w_attn dtype (DMA-bound input path)


def _emit(nc, tc, ctx):
    xT = nc.dram_tensor("xT", [C, T], IN_DT, kind="ExternalInput").ap()
    wqkvT = nc.dram_tensor("wqkvT", [C, 384], IN_DT, kind="ExternalInput").ap()
    bqkv = nc.dram_tensor("bqkv", [128, 3], F32, kind="ExternalInput").ap()
    wpT = nc.dram_tensor("wpT", [128, C], MM_DT, kind="ExternalInput").ap()
    bp = nc.dram_tensor("bp", [128, 4], F32, kind="ExternalInput").ap()
    mask01 = nc.dram_tensor("mask01", [128, 128], F32, kind="ExternalInput").ap()
    ident = nc.dram_tensor("ident", [128, 128], F32, kind="ExternalInput").ap()
    e64 = nc.dram_tensor("e64", [128, 64], MM_DT, kind="ExternalInput").ap()
    outT = nc.dram_tensor("outT", [C, T], BF16, kind="ExternalOutput").ap()

    consts = ctx.enter_context(tc.tile_pool(name="consts", bufs=1))
    big = ctx.enter_context(tc.tile_pool(name="big", bufs=1))
    vt_pool = ctx.enter_context(tc.tile_pool(name="vt", bufs=2))
    pt_pool = ctx.enter_context(tc.tile_pool(name="pt", bufs=4))
    osb_pool = ctx.enter_context(tc.tile_pool(name="osb", bufs=3))
    rc_pool = ctx.enter_context(tc.tile_pool(name="rc", bufs=3))
    yn_pool = ctx.enter_context(tc.tile_pool(name="yn", bufs=6))
    ob_pool = ctx.enter_context(tc.tile_pool(name="ob", bufs=4))
    # PSUM: 8 banks = stage 2x[128,1024] (4) + o 2x[128,512] (2) + sm 2x (2)
    ps_stage = ctx.enter_context(tc.tile_pool(name="ps_st", bufs=2, space="PSUM"))
    ps_o = ctx.enter_context(tc.tile_pool(name="ps_o", bufs=2, space="PSUM"))
    ps_sm = ctx.enter_context(tc.tile_pool(name="ps_sm", bufs=2, space="PSUM"))

    # --- constants (id_sb first: it gates the PE warm-up matmuls) ---
    id_sb = consts.tile([128, 128], F32, name="id_sb")
    nc.sync.dma_start(out=id_sb, in_=ident)
    w_sb = consts.tile([128, 4, 384], IN_DT, name="w_sb")
    nc.sync.dma_start(out=w_sb, in_=wqkvT.rearrange("(c p) m -> p c m", p=128))
    bqkv_sb = consts.tile([128, 3], F32, name="bqkv_sb")
    nc.sync.dma_start(out=bqkv_sb, in_=bqkv)

    # x resident in SBUF via big DMAs issued up-front: streaming x during the
    # attention phase inflates every SBUF access ~20% (measured), so the
    # input burst is confined to the cheap QKV ramp. First wave is just the
    # 0.5 MB that QKV(0) needs, so compute starts early.
    xfull = big.tile([128, 4, T], IN_DT, name="xfull")
    for cc in range(4):
        nc.sync.dma_start(out=xfull[:, cc, 0:TQ],
                          in_=xT[cc * 128:(cc + 1) * 128, 0:TQ])
    for cc in range(4):
        nc.sync.dma_start(out=xfull[:, cc, TQ:2048],
                          in_=xT[cc * 128:(cc + 1) * 128, TQ:2048])

    wp_sb = consts.tile([128, C], MM_DT, name="wp_sb")
    nc.sync.dma_start(out=wp_sb, in_=wpT)
    bp_sb = consts.tile([128, 4], F32, name="bp_sb")
    nc.sync.dma_start(out=bp_sb, in_=bp)
    mask_sb = consts.tile([128, 128], F32, name="mask_sb")
    nc.sync.dma_start(out=mask_sb, in_=mask01)
    e64_sb = consts.tile([128, 64], MM_DT, name="e64_sb")
    nc.sync.dma_start(out=e64_sb, in_=e64)

    for cc in range(4):
        nc.sync.dma_start(out=xfull[:, cc, 2048:T],
                          in_=xT[cc * 128:(cc + 1) * 128, 2048:T])

    # PE warm-up: ~4.5us of throwaway matmuls flips the HAM clock gate to
    # 2.4 GHz before the real QKV matmuls arrive (transpose-mode does not
    # count as PE-busy for HAM, so these are plain matmuls).
    for wi in range(10):
        wu = ps_sm.tile([128, 128], F32, name="wu", tag="sm")
        nc.tensor.matmul(wu, lhsT=id_sb, rhs=id_sb, start=True, stop=True)

    # per-head q (zero-padded to 128 partitions), shared k, natural-layout v
    qT0 = big.tile([128, T], MM_DT, name="qT0")
    qT1 = big.tile([128, T], MM_DT, name="qT1")
    kT_sb = big.tile([128, T], MM_DT, name="kT_sb")
    nc.vector.memset(qT0[64:128, :].bitcast(mybir.dt.uint32), 0)
    nc.gpsimd.memset(qT1[0:64, :].bitcast(mybir.dt.uint32), 0)
    # V per 128-key block with ones columns: [j, 0:64]=v_h0, 64=ones,
    # [65:129]=v_h1, 129=ones. memset, not DMA: a stride-0 broadcast DMA is
    # 4096 4-byte packets that hog the DMA queue for ~40us.
    v_all = big.tile([128, NJ, 130], MM_DT, name="v_all")
    one_bits = 0x3F800000
    nc.gpsimd.memset(v_all[:, :, 64:65].bitcast(mybir.dt.uint32), one_bits)
    nc.gpsimd.memset(v_all[:, :, 129:130].bitcast(mybir.dt.uint32), one_bits)

    # --- QKV projection for one i-tile (generator: yields between chunks) ---
    def emit_qkv(t):
        t0 = t * TQ
        for m in range(3):  # q, k, v
            ps = ps_sm.tile([128, TQ], F32, name="qkv_ps", tag="sm")
            for c in range(4):
                nc.tensor.matmul(
                    ps,
                    lhsT=w_sb[:, c, m * 128:(m + 1) * 128],
                    rhs=xfull[:, c, t0:t0 + TQ],
                    start=(c == 0),
                    stop=(c == 3),
                )
            if m == 0:
                # q scale (1/sqrt(D)) folded into wqkvT/bqkv on the host
                nc.vector.tensor_scalar_add(
                    qT0[0:64, t0:t0 + TQ], ps[0:64, :], bqkv_sb[0:64, 0:1])
                nc.vector.tensor_scalar_add(
                    qT1[64:128, t0:t0 + TQ], ps[64:128, :], bqkv_sb[64:128, 0:1])
            elif m == 1:
                nc.vector.tensor_scalar_add(
                    kT_sb[:, t0:t0 + TQ], ps, bqkv_sb[:, 1:2])
            else:
                vt = vt_pool.tile([128, TQ], F32, name="vt", tag="vt")
                nc.vector.tensor_scalar_add(vt, ps, bqkv_sb[:, 2:3])
                for s in range(4):
                    n = t * 4 + s
                    tp = ps_sm.tile([128, 128], F32, name="tp", tag="sm")
                    # fp32 PE transpose (exact): [vdim, t]^T -> [t, vdim]
                    nc.tensor.transpose(tp, vt[:, s * 128:(s + 1) * 128], id_sb)
                    dst = v_all[:, n, :].rearrange(
                        "p (g e) -> p g e", g=2, e=65)[:, :, 0:64]
                    src = tp.rearrange("p (g e) -> p g e", g=2, e=64)
                    nc.vector.tensor_copy(dst, src)
            yield

    qkv_gens = [emit_qkv(t) for t in range(NI)]

    def drive(gen):
        if gen is not None:
            try:
                next(gen)
            except StopIteration:
                pass

    def finish(gen):
        if gen is not None:
            for _ in gen:
                pass

    finish(qkv_gens[0])  # i-tile 0 needed immediately

    # which QKV emission to interleave into each (half, h, i_t) block
    def gen_for(half, h, i_t):
        if half == 0 and h == 0 and i_t < 3:
            return qkv_gens[i_t + 1]
        if half == 0 and h == 1:
            return qkv_gens[4 + i_t]
        return None

    # --- attention: i_t-outer, J-pairs staged into one wide ACT ---
    pending_tail = [None]

    def flush_tail():
        if pending_tail[0] is not None:
            pending_tail[0]()
            pending_tail[0] = None

    yns = {}
    for half in range(2):
        for h in range(2):
            qT_h = qT0 if h == 0 else qT1
            for i_t in range(half * 4, half * 4 + 4):
                i0 = i_t * TQ
                nJ = 4 * i_t + 4
                gen = gen_for(half, h, i_t)
                o = ps_o.tile([128, TQ], F32, name="o_ps", tag="o")
                for Ja in range(0, nJ, 2):
                    drive(gen)
                    stage = ps_stage.tile([128, 1024], F32, name="st", tag="st")
                    metas, off = [], 0
                    for J in (Ja, Ja + 1):
                        r = max(0, J * TJ - i0)
                        w = TQ - r
                        nc.tensor.matmul(
                            stage[:, off:off + w],
                            lhsT=kT_sb[:, J * TJ:(J + 1) * TJ],
                            rhs=qT_h[:, i0 + r:i0 + TQ],
                            start=True, stop=True,
                        )
                        metas.append((J, r, off, w))
                        off += w
                    pt = pt_pool.tile([128, 1024], MM_DT, name="pt", tag="pt")
                    nc.scalar.activation(pt[:, 0:off], stage[:, 0:off], AF.Exp)
                    for (J, r, o_, w) in metas:
                        if J >= 4 * i_t:  # diag block: in-block triangle mask
                            nc.vector.tensor_mul(
                                pt[:, o_:o_ + 128], pt[:, o_:o_ + 128], mask_sb)
                    for (J, r, o_, w) in metas:
                        v_lhs = (v_all[:, J, 0:65] if h == 0
                                 else v_all[:, J, 65:130])
                        nc.tensor.matmul(
                            o[0:65, r:TQ],
                            lhsT=v_lhs,
                            rhs=pt[:, o_:o_ + w],
                            start=(J == 0),
                            stop=(J == nJ - 1),
                        )
                    if Ja == 0:
                        flush_tail()
                finish(gen)

                def make_tail(h=h, i_t=i_t, o=o, i0=i0):
                    def tail():
                        # softmax division: broadcast denominator row via a
                        # one-hot matmul, reciprocal, multiply; evict O' from
                        # PSUM to SBUF first so the bank frees early.
                        o_sb = osb_pool.tile([128, TQ], MM_DT, name="o_sb",
                                             tag="osb")
                        nc.vector.tensor_copy(o_sb[0:65, :], o[0:65, :])
                        den = ps_sm.tile([128, TQ], F32, name="den", tag="sm")
                        nc.tensor.matmul(
                            den[0:64, :], lhsT=e64_sb[0:65, :],
                            rhs=o_sb[0:65, :], start=True, stop=True)
                        rc = rc_pool.tile([64, TQ], F32, name="rc", tag="rc")
                        nc.vector.reciprocal_approx_fast(out=rc, in_=den[0:64, :])
                        if h == 0:
                            yn = yn_pool.tile([128, TQ], MM_DT, name="yn",
                                              tag="yn")
                            yns[i_t] = yn
                        else:
                            yn = yns[i_t]
                        nc.vector.tensor_mul(
                            yn[h * 64:(h + 1) * 64, :], o_sb[0:64, :], rc)
                        if h == 1:
                            # output projection for this i_t (both heads)
                            for mc in range(4):
                                po = ps_sm.tile([128, TQ], F32, name="po",
                                                tag="sm")
                                nc.tensor.matmul(
                                    po, lhsT=wp_sb[:, mc * 128:(mc + 1) * 128],
                                    rhs=yn, start=True, stop=True)
                                ob = ob_pool.tile([128, TQ], BF16, name="ob",
                                                  tag="ob")
                                nc.vector.tensor_scalar_add(
                                    ob, po, bp_sb[:, mc:mc + 1])
                                nc.sync.dma_start(
                                    out=outT[mc * 128:(mc + 1) * 128,
                                             i0:i0 + TQ],
                                    in_=ob)
                    return tail

                pending_tail[0] = make_tail()
    flush_tail()


_CACHED_NC = None


def _build_program():
    global _CACHED_NC
    if _CACHED_NC is not None:
        return _CACHED_NC
    from contextlib import ExitStack
    nc = bacc.Bacc("TRN2", target_bir_lowering=False, debug=False,
                   num_devices=N_CORES)
    with tile.TileContext(nc) as tc:
        with ExitStack() as ctx:
            _emit(nc, tc, ctx)
    nc.compile()
    _CACHED_NC = nc
    return nc


def _host_inputs(x, w_attn, b_attn, w_proj, b_proj):
    """Build the 8 per-core input maps."""
    import ml_dtypes
    innp = ml_dtypes.bfloat16
    x = np.asarray(x, dtype=np.float32)
    w_attn = np.asarray(w_attn, dtype=np.float32)
    b_attn = np.asarray(b_attn, dtype=np.float32)
    w_proj = np.asarray(w_proj, dtype=np.float32)
    b_proj = np.asarray(b_proj, dtype=np.float32)

    scale = np.float32(1.0 / np.sqrt(D))
    mask = np.triu(np.ones((128, 128), dtype=np.float32))  # keep jj <= ii
    ident = np.eye(128, dtype=np.float32)
    e64 = np.zeros((128, 64), dtype=np.float32)
    e64[64, :] = 1.0

    xT_b = [np.ascontiguousarray(x[b].T).astype(innp) for b in range(B)]

    in_maps = []
    for core in range(N_CORES):
        b, hp = divmod(core, 4)
        r0 = 2 * hp * 64  # first row of this core's head-pair slice
        qr = w_attn[r0:r0 + 128] * scale
        kr = w_attn[C + r0:C + r0 + 128]
        vr = w_attn[2 * C + r0:2 * C + r0 + 128]
        wqkvT = np.ascontiguousarray(np.concatenate([qr, kr, vr], axis=0).T)
        bq = b_attn[r0:r0 + 128] * scale
        bk = b_attn[C + r0:C + r0 + 128]
        bv = b_attn[2 * C + r0:2 * C + r0 + 128]
        bqkv = np.ascontiguousarray(np.stack([bq, bk, bv], axis=1))
        wpT = np.ascontiguousarray(w_proj[:, r0:r0 + 128].T)
        if hp == 0:
            bpc = np.ascontiguousarray(b_proj.reshape(4, 128).T)
        else:
            bpc = np.zeros((128, 4), dtype=np.float32)
        in_maps.append({
            "xT": xT_b[b],
            "wqkvT": wqkvT.astype(innp),
            "bqkv": bqkv,
            "wpT": wpT.astype(np.float32),
            "bp": bpc,
            "mask01": mask,
            "ident": ident,
            "e64": e64,
        })
    return in_maps


def _gather(results):
    out = np.empty((B, T, C), dtype=np.float32)
    for b in range(B):
        acc = results[b * 4]["outT"].astype(np.float32)
        for hp in range(1, 4):
            acc = acc + results[b * 4 + hp]["outT"].astype(np.float32)
        out[b] = acc.T
    return out


def kernel(x, w_attn, b_attn, w_proj, b_proj, _run_kwargs=None):
    nc = _build_program()
    in_maps = _host_inputs(x, w_attn, b_attn, w_proj, b_proj)
    kw = dict(_run_kwargs or {})
    res = bass_utils.run_bass_kernel_spmd(nc, in_maps,
                                          core_ids=list(range(N_CORES)), **kw)
    out = _gather(res.results)
    if _run_kwargs is not None:
        kernel.last_result = res
    return out
